# revision 19
# baseline (speedup 1.0000x reference)
"""Trainium2 Bass kernel for MoE (nn_MoE_75170517615144).

Strategy: data-parallel over tokens (1024 tokens/core x 8 cores), all 16
routed experts + shared expert computed per core on its token shard.
Everything runs feature-major ([feature, token] layout) so the FFN needs no
transposes; matmuls use float32r (full-rate fp32 on the PE).

v1 = masked-dense: every expert computed for every token, combine weights
zero out unselected experts (matches reference semantics directly).

Host side: shards x, pre-transposes weights, feeds 8 identical-program cores
via run_bass_kernel_spmd; output is concatenated + transposed back.
"""
import sys

sys.path.insert(0, "/opt/trn_rl_repo")

import numpy as np

DIM = 1024
INTER = 1024
E = 16          # routed experts
NE = 18         # + shared expert split into 2 pseudo-experts
SINTER = 2048
T = 8192
NCORES = 8
TSH = T // NCORES   # tokens per core
P = 128
KD = DIM // P       # 8 contraction chunks

_CACHE = {}


def _build_bass():
    import concourse.bacc as bacc
    import concourse.tile as tile
    import concourse.mybir as mybir
    from concourse.masks import make_identity

    f32 = mybir.dt.float32
    f32r = mybir.dt.float32r
    AF = mybir.ActivationFunctionType
    OP = mybir.AluOpType
    AX = mybir.AxisListType

    nc = bacc.Bacc("TRN2", target_bir_lowering=False, debug=False)

    xT_d = nc.dram_tensor("xT", [DIM, TSH], f32, kind="ExternalInput")
    w13_d = nc.dram_tensor("w13", [NE, DIM, 2 * INTER], f32, kind="ExternalInput")
    w2_d = nc.dram_tensor("w2", [NE, INTER, DIM], f32, kind="ExternalInput")
    gwT_d = nc.dram_tensor("gwT", [DIM, E], f32, kind="ExternalInput")
    b1_d = nc.dram_tensor("b1H", [P, NE * 8], f32, kind="ExternalInput")
    b3_d = nc.dram_tensor("b3H", [P, NE * 8], f32, kind="ExternalInput")
    b2_d = nc.dram_tensor("b2e", [16, DIM], f32, kind="ExternalInput")
    sb2_d = nc.dram_tensor("sb2H", [P, KD], f32, kind="ExternalInput")
    out_d = nc.dram_tensor("out", [DIM, TSH], f32, kind="ExternalOutput")
    cw_dram = nc.dram_tensor("cw_scratch", [E, TSH], f32)  # internal scratch

    with tile.TileContext(nc) as tc:
        persist = tc.alloc_tile_pool(name="persist", bufs=1)
        setup = tc.alloc_tile_pool(name="setup", bufs=1)
        xtmp = tc.alloc_tile_pool(name="xtmp", bufs=2)
        wpool = tc.alloc_tile_pool(name="wpool", bufs=2)
        wrpool = tc.alloc_tile_pool(name="wrpool", bufs=2)
        w2pool = tc.alloc_tile_pool(name="w2pool", bufs=4)
        w2rpool = tc.alloc_tile_pool(name="w2rpool", bufs=4)
        misc = tc.alloc_tile_pool(name="misc", bufs=2)
        g3pool = tc.alloc_tile_pool(name="g3pool", bufs=2)
        g1pool = tc.alloc_tile_pool(name="g1pool", bufs=2)
        hxpool = tc.alloc_tile_pool(name="hxpool", bufs=1)
        cwepool = tc.alloc_tile_pool(name="cwepool", bufs=2)
        ph = tc.alloc_tile_pool(name="ph", bufs=2, space="PSUM")
        py = tc.alloc_tile_pool(name="py", bufs=2, space="PSUM")

        # ---------- load x (transposed), round to f32r, and gate ----------
        # The gate matmul runs in plain fp32 (exact) off the staging tiles so
        # the top-4 selection tie-breaks identically to the fp32 reference.
        gw = setup.tile([P, KD, E], f32, tag="gw")
        nc.sync.dma_start(gw[:], gwT_d.ap().rearrange("(ko p) e -> p ko e", p=P))

        xr = persist.tile([P, KD, TSH], f32r, tag="xr")
        lg_ps = ph.tile([E, TSH], f32, tag="h")
        for k in range(KD):
            xt = xtmp.tile([P, TSH], f32, tag="xt")
            nc.sync.dma_start(xt[:], xT_d.ap()[k * P:(k + 1) * P, :])
            nc.vector.tensor_copy(xr[:, k], xt[:])
            nc.tensor.matmul(lg_ps[:, :512], gw[:, k], xt[:, :512],
                             start=(k == 0), stop=(k == KD - 1))
            nc.tensor.matmul(lg_ps[:, 512:], gw[:, k], xt[:, 512:],
                             start=(k == 0), stop=(k == KD - 1))
        lg = setup.tile([E, TSH], f32, tag="lg")
        nc.vector.tensor_copy(lg[:], lg_ps[:])

        id16 = persist.tile([16, 16], f32, tag="id16")
        make_identity(nc, id16[:])
        id128 = persist.tile([P, P], f32, tag="id128")
        make_identity(nc, id128[:])

        # cw16r: combine weights, expert-major.  rhs of the bias-combine
        # matmul; also written to DRAM for the per-expert broadcast loads.
        cw16r = persist.tile([16, TSH], f32r, tag="cw16r")

        nblk = TSH // P
        for b in range(nblk):
            ltp = py.tile([P, 16], f32, tag="y")
            nc.tensor.transpose(ltp[:], lg[:, b * P:(b + 1) * P], id16[:])
            lt = misc.tile([P, 16], f32, tag="lt")
            nc.vector.tensor_copy(lt[:], ltp[:])
            # softmax over the 16 experts (free axis)
            nm = misc.tile([P, 1], f32, tag="nm")
            nc.vector.tensor_reduce(nm[:], lt[:], axis=AX.X, op=OP.max, negate=True)
            es = misc.tile([P, 16], f32, tag="es")
            nc.scalar.activation(es[:], lt[:], AF.Exp, bias=nm[:])
            sm = misc.tile([P, 1], f32, tag="sm")
            nc.vector.tensor_reduce(sm[:], es[:], axis=AX.X, op=OP.add)
            rs = misc.tile([P, 1], f32, tag="rs")
            nc.vector.reciprocal(rs[:], sm[:])
            sS = misc.tile([P, 16], f32, tag="sS")
            nc.vector.tensor_scalar_mul(sS[:], es[:], rs[:])
            # top-4 mask
            t8 = misc.tile([P, 8], f32, tag="t8")
            nc.vector.max(t8[:], sS[:])
            mk = misc.tile([P, 16], f32, tag="mk")
            nc.vector.tensor_scalar(mk[:], sS[:], t8[:, 3:4], None, op0=OP.is_ge)
            cwb = misc.tile([P, 16], f32, tag="cwb")
            nc.vector.tensor_mul(cwb[:], sS[:], mk[:])
            # transpose back to expert-major
            ctp = py.tile([16, P], f32, tag="y")
            nc.tensor.transpose(ctp[:], cwb[:], id128[:])
            nc.vector.tensor_copy(cw16r[:, b * P:(b + 1) * P], ctp[:])

        # stash cw rows in DRAM for cheap partition-broadcast reads
        nc.sync.dma_start(cw_dram.ap(), cw16r[:].bitcast(f32))

        # ---------- bias-combine init of y accumulator ----------
        b2s = setup.tile([16, DIM], f32, tag="b2s")
        nc.sync.dma_start(b2s[:], b2_d.ap())
        b2r = setup.tile([16, DIM], f32r, tag="b2r")
        nc.vector.tensor_copy(b2r[:], b2s[:])
        sb2s = persist.tile([P, KD], f32, tag="sb2s")
        nc.sync.dma_start(sb2s[:], sb2_d.ap())

        y_acc = persist.tile([P, KD, TSH], f32, tag="yacc")
        for d in range(KD):
            yp = py.tile([P, TSH], f32, tag="y")
            nc.tensor.matmul(yp[:, :512], b2r[:, d * P:(d + 1) * P],
                             cw16r[:, :512], start=True, stop=True)
            nc.tensor.matmul(yp[:, 512:], b2r[:, d * P:(d + 1) * P],
                             cw16r[:, 512:], start=True, stop=True)
            nc.vector.tensor_copy(y_acc[:, d], yp[:])

        # ---------- biases ----------
        b1s = persist.tile([P, NE * 8], f32, tag="b1s")
        nc.sync.dma_start(b1s[:], b1_d.ap())
        b3s = persist.tile([P, NE * 8], f32, tag="b3s")
        nc.sync.dma_start(b3s[:], b3_d.ap())

        # ---------- expert loop (16 routed + 2 shared halves) ----------
        for e in range(NE):
            routed = e < E
            if routed:
                cwe = cwepool.tile([P, TSH], f32, tag="cwe")
                nc.sync.dma_start(
                    cwe[:], cw_dram.ap()[e:e + 1, :].to_broadcast((P, TSH)))

            hx = hxpool.tile([P, 8, TSH], f32r, tag="hx")
            for ic in range(8):
                # stream w1 and w3 column-blocks for this i-chunk
                hps = []
                for w in range(2):
                    col = w * INTER + ic * P
                    wc = wpool.tile([P, KD, P], f32, tag="w13")
                    nc.sync.dma_start(
                        wc[:],
                        w13_d.ap()[e, :, col:col + P]
                        .rearrange("(ko p) i -> p ko i", p=P))
                    wr = wrpool.tile([P, KD, P], f32r, tag="w13r")
                    nc.gpsimd.tensor_copy(wr[:], wc[:])
                    hp = ph.tile([P, TSH], f32, tag="h")
                    for k in range(KD):
                        nc.tensor.matmul(hp[:, :512], wr[:, k], xr[:, k, :512],
                                         start=(k == 0), stop=(k == KD - 1))
                        nc.tensor.matmul(hp[:, 512:], wr[:, k], xr[:, k, 512:],
                                         start=(k == 0), stop=(k == KD - 1))
                    hps.append(hp)
                bcol = e * 8 + ic
                g1 = g1pool.tile([P, TSH], f32, tag="g1")
                nc.scalar.activation(g1[:], hps[0][:], AF.Silu,
                                     bias=b1s[:, bcol:bcol + 1])
                g3 = g3pool.tile([P, TSH], f32, tag="g3")
                nc.scalar.activation(g3[:], hps[1][:], AF.Identity,
                                     bias=b3s[:, bcol:bcol + 1])
                nc.vector.tensor_mul(hx[:, ic], g1[:], g3[:])
                if routed:
                    nc.vector.tensor_mul(hx[:, ic], hx[:, ic], cwe[:])

            for d in range(KD):
                yp = py.tile([P, TSH], f32, tag="y")
                for i in range(8):
                    w2c = w2pool.tile([P, P], f32, tag="w2")
                    nc.sync.dma_start(
                        w2c[:], w2_d.ap()[e, i * P:(i + 1) * P, d * P:(d + 1) * P])
                    w2r = w2rpool.tile([P, P], f32r, tag="w2r")
                    nc.gpsimd.tensor_copy(w2r[:], w2c[:])
                    nc.tensor.matmul(yp[:, :512], w2r[:], hx[:, i, :512],
                                     start=(i == 0), stop=(i == 7))
                    nc.tensor.matmul(yp[:, 512:], w2r[:], hx[:, i, 512:],
                                     start=(i == 0), stop=(i == 7))
                nc.vector.tensor_add(y_acc[:, d], y_acc[:, d], yp[:])

        # ---------- output (+ shared-expert output bias sb2) ----------
        for d in range(KD):
            nc.vector.tensor_scalar(y_acc[:, d], y_acc[:, d],
                                    sb2s[:, d:d + 1], None, op0=OP.add)
            nc.sync.dma_start(out_d.ap()[d * P:(d + 1) * P, :], y_acc[:, d])

        for pool in reversed((persist, setup, xtmp, wpool, wrpool, w2pool,
                              w2rpool, misc, g3pool, g1pool, hxpool, cwepool,
                              ph, py)):
            pool.release()

    nc.compile()
    return nc




CAP = 384           # padded dispatch capacity per expert (mean 256, 9 sigma)
NC3 = CAP // 128
NB = TSH // P       # token tiles per core


def _build_bass_v2():
    """Sparse-dispatch variant: top-4 routing drives one-hot dispatch
    matrices built on the DVE; gather, per-slot gating, and combine all run
    as PE matmuls.  Routed experts compute on CAP=384 gathered tokens
    instead of all 1024."""
    import concourse.bacc as bacc
    import concourse.tile as tile
    import concourse.mybir as mybir
    from concourse.masks import make_identity

    f32 = mybir.dt.float32
    f32r = mybir.dt.float32r
    i32 = mybir.dt.int32
    AF = mybir.ActivationFunctionType
    OP = mybir.AluOpType
    AX = mybir.AxisListType

    nc = bacc.Bacc("TRN2", target_bir_lowering=False, debug=False)

    xT_d = nc.dram_tensor("xT", [DIM, TSH], f32, kind="ExternalInput")
    xrow_d = nc.dram_tensor("xrow", [TSH, DIM], f32, kind="ExternalInput")
    w13_d = nc.dram_tensor("w13", [NE, DIM, 2 * INTER], f32, kind="ExternalInput")
    w2_d = nc.dram_tensor("w2", [NE, INTER, DIM], f32, kind="ExternalInput")
    gwT_d = nc.dram_tensor("gwT", [DIM, E], f32, kind="ExternalInput")
    b1_d = nc.dram_tensor("b1H", [P, NE * 8], f32, kind="ExternalInput")
    b3_d = nc.dram_tensor("b3H", [P, NE * 8], f32, kind="ExternalInput")
    b2_d = nc.dram_tensor("b2e", [16, DIM], f32, kind="ExternalInput")
    sb2_d = nc.dram_tensor("sb2H", [P, KD], f32, kind="ExternalInput")
    out_d = nc.dram_tensor("outTok", [TSH, DIM], f32, kind="ExternalOutput")
    slot_dram = nc.dram_tensor("slot_scratch", [E, TSH], f32)
    cw_dram = nc.dram_tensor("cw_scratch2", [E, TSH], f32)

    with tile.TileContext(nc) as tc:
        persist = tc.alloc_tile_pool(name="persist", bufs=1)
        setup = tc.alloc_tile_pool(name="setup", bufs=1)
        xtmp = tc.alloc_tile_pool(name="xtmp", bufs=1)
        wpool = tc.alloc_tile_pool(name="wpool", bufs=2)
        wrpool = tc.alloc_tile_pool(name="wrpool", bufs=2)
        w2pool = tc.alloc_tile_pool(name="w2pool", bufs=4)
        w2rpool = tc.alloc_tile_pool(name="w2rpool", bufs=4)
        misc = tc.alloc_tile_pool(name="misc", bufs=2)
        g3pool = tc.alloc_tile_pool(name="g3pool", bufs=2)
        g1pool = tc.alloc_tile_pool(name="g1pool", bufs=2)
        hxpool = tc.alloc_tile_pool(name="hxpool", bufs=1)
        cpool = tc.alloc_tile_pool(name="cpool", bufs=1)
        xgpool = tc.alloc_tile_pool(name="xgpool", bufs=1)
        ytpool = tc.alloc_tile_pool(name="ytpool", bufs=1)
        slbpool = tc.alloc_tile_pool(name="slbpool", bufs=1)
        cslpool = tc.alloc_tile_pool(name="cslpool", bufs=3)
        xshpool = tc.alloc_tile_pool(name="xshpool", bufs=1)
        ph = tc.alloc_tile_pool(name="ph", bufs=2, space="PSUM")
        py = tc.alloc_tile_pool(name="py", bufs=2, space="PSUM")
        pg = tc.alloc_tile_pool(name="pg", bufs=2, space="PSUM")
        ptr = tc.alloc_tile_pool(name="ptr", bufs=2, space="PSUM")

        # ---------- load x row-major (rounded) + gate logits (exact fp32) ---
        gw = setup.tile([P, KD, E], f32, tag="gw")
        nc.sync.dma_start(gw[:], gwT_d.ap().rearrange("(ko p) e -> p ko e", p=P))

        xw_r = persist.tile([P, NB, DIM], f32r, tag="xwr")
        for b in range(NB):
            xt = xtmp.tile([P, DIM], f32, tag="xt")
            nc.sync.dma_start(xt[:], xrow_d.ap()[b * P:(b + 1) * P, :])
            nc.vector.tensor_copy(xw_r[:, b], xt[:])

        lg = setup.tile([E, TSH], f32, tag="lg")
        lg_hs = [ph.tile([E, 512], f32, tag="h2", name=f"lg_h{_h}")
                 for _h in range(2)]
        for k in range(KD):
            xtc = xtmp.tile([P, TSH], f32, tag="xt")
            nc.sync.dma_start(xtc[:], xT_d.ap()[k * P:(k + 1) * P, :])
            for h in range(2):
                nc.tensor.matmul(lg_hs[h][:], gw[:, k],
                                 xtc[:, h * 512:(h + 1) * 512],
                                 start=(k == 0), stop=(k == KD - 1))
        for h in range(2):
            nc.vector.tensor_copy(lg[:, h * 512:(h + 1) * 512], lg_hs[h][:])

        id16 = persist.tile([16, 16], f32, tag="id16")
        make_identity(nc, id16[:])
        id128 = persist.tile([P, P], f32, tag="id128")
        make_identity(nc, id128[:])
        id128r = persist.tile([P, P], f32r, tag="id128r")
        nc.vector.tensor_copy(id128r[:], id128[:])

        cw16r = persist.tile([16, TSH], f32r, tag="cw16r")
        cwTok = persist.tile([P, NB, 16], f32, tag="cwTok")

        for b in range(NB):
            ltp = ptr.tile([P, 16], f32, tag="tr")
            nc.tensor.transpose(ltp[:], lg[:, b * P:(b + 1) * P], id16[:])
            lt = misc.tile([P, 16], f32, tag="lt")
            nc.vector.tensor_copy(lt[:], ltp[:])
            nm = misc.tile([P, 1], f32, tag="nm")
            nc.vector.tensor_reduce(nm[:], lt[:], axis=AX.X, op=OP.max, negate=True)
            es = misc.tile([P, 16], f32, tag="es")
            nc.scalar.activation(es[:], lt[:], AF.Exp, bias=nm[:])
            sm = misc.tile([P, 1], f32, tag="sm")
            nc.vector.tensor_reduce(sm[:], es[:], axis=AX.X, op=OP.add)
            rs = misc.tile([P, 1], f32, tag="rs")
            nc.vector.reciprocal(rs[:], sm[:])
            sS = misc.tile([P, 16], f32, tag="sS")
            nc.vector.tensor_scalar_mul(sS[:], es[:], rs[:])
            t8 = misc.tile([P, 8], f32, tag="t8")
            nc.vector.max(t8[:], sS[:])
            mk = misc.tile([P, 16], f32, tag="mk")
            nc.vector.tensor_scalar(mk[:], sS[:], t8[:, 3:4], None, op0=OP.is_ge)
            nc.vector.tensor_mul(cwTok[:, b], sS[:], mk[:])
            ctp = ptr.tile([16, P], f32, tag="tr")
            nc.tensor.transpose(ctp[:], cwTok[:, b], id128[:])
            nc.vector.tensor_copy(cw16r[:, b * P:(b + 1) * P], ctp[:])

        # ---------- slot machinery (expert-major) ----------
        cwf = cw16r[:].bitcast(f32)
        maskT = setup.tile([16, TSH], f32, tag="maskT")
        nc.vector.tensor_scalar(maskT[:], cwf, 0.0, None, op0=OP.is_gt)
        incl = setup.tile([16, TSH], f32, tag="incl")
        nc.vector.tensor_tensor_scan(incl[:], maskT[:], maskT[:], 0.0,
                                     op0=OP.add, op1=OP.bypass)
        io16 = setup.tile([16, 1], i32, tag="io16")
        nc.gpsimd.iota(io16[:], pattern=[[0, 1]], base=1, channel_multiplier=CAP)
        io16f = setup.tile([16, 1], f32, tag="io16f")
        nc.vector.tensor_copy(io16f[:], io16[:])
        nc.vector.tensor_sub(incl[:], incl[:], maskT[:])
        nc.vector.tensor_scalar(incl[:], incl[:], io16f[:], None, op0=OP.add)
        sl1 = incl
        nc.vector.tensor_mul(sl1[:], sl1[:], maskT[:])   # slot+1 or 0
        nc.sync.dma_start(slot_dram.ap(), sl1[:])
        nc.sync.dma_start(cw_dram.ap(), cw16r[:].bitcast(f32))

        # slot+1 token-major [128, NB, 16]
        slotTok = persist.tile([P, NB, 16], f32, tag="slotTok")
        for b in range(NB):
            stp = ptr.tile([P, 16], f32, tag="tr")
            nc.tensor.transpose(stp[:], sl1[:, b * P:(b + 1) * P], id16[:])
            nc.vector.tensor_copy(slotTok[:, b], stp[:])

        ioP = persist.tile([P, 1], i32, tag="ioP")
        nc.gpsimd.iota(ioP[:], pattern=[[0, 1]], base=0, channel_multiplier=1)
        ioPf = persist.tile([P, 1], f32, tag="ioPf")
        nc.vector.tensor_copy(ioPf[:], ioP[:])
        ioJ = persist.tile([P, CAP], i32, tag="ioJ")
        nc.gpsimd.iota(ioJ[:], pattern=[[1, CAP]], base=1, channel_multiplier=0)
        ioJf = persist.tile([P, CAP], f32, tag="ioJf")
        nc.vector.tensor_copy(ioJf[:], ioJ[:])

        # ---------- biases ----------
        b2s = setup.tile([16, DIM], f32, tag="b2s")
        nc.sync.dma_start(b2s[:], b2_d.ap())
        b2r = setup.tile([16, DIM], f32r, tag="b2r")
        nc.vector.tensor_copy(b2r[:], b2s[:])
        sb2s = persist.tile([P, KD], f32, tag="sb2s")
        nc.sync.dma_start(sb2s[:], sb2_d.ap())
        b1s = persist.tile([P, NE * 8], f32, tag="b1s")
        nc.sync.dma_start(b1s[:], b1_d.ap())
        b3s = persist.tile([P, NE * 8], f32, tag="b3s")
        nc.sync.dma_start(b3s[:], b3_d.ap())

        # ---------- bias-combine init of token-major accumulator ----------
        acc = persist.tile([P, NB, DIM], f32, tag="acc")
        for b in range(NB):
            for h in range(2):
                ap_ = py.tile([P, 512], f32, tag="y2")
                nc.tensor.matmul(ap_[:], cw16r[:, b * P:(b + 1) * P],
                                 b2r[:, h * 512:(h + 1) * 512],
                                 start=True, stop=True)
                nc.vector.tensor_copy(acc[:, b, h * 512:(h + 1) * 512], ap_[:])

        # ---------- routed experts ----------
        for e in range(E):
            # one-hot dispatch (token-major): CeT[t, j] = (j+e*CAP+1 == slot+1)
            CeT = cpool.tile([P, NB, CAP], f32r, tag="CeT")
            for b in range(NB):
                nc.vector.tensor_scalar(CeT[:, b], ioJf[:],
                                        slotTok[:, b, e:e + 1],
                                        float(-e * CAP),
                                        op0=OP.subtract, op1=OP.is_equal)

            # gather: xgr[d_chunk] = sum_b xrow_b(d_chunk)^T @ CeT_b
            xgr = xgpool.tile([P, KD, CAP], f32r, tag="xgr")
            for d in range(KD):
                gp = pg.tile([P, CAP], f32, tag="g")
                for b in range(NB):
                    nc.tensor.matmul(gp[:], xw_r[:, b, d * P:(d + 1) * P],
                                     CeT[:, b], start=(b == 0),
                                     stop=(b == NB - 1))
                nc.scalar.copy(xgr[:, d], gp[:])

            # FFN layer 1 on gathered tokens
            hx = hxpool.tile([P, 8, CAP], f32r, tag="hx")
            for ic in range(8):
                hps = []
                for w in range(2):
                    col = w * INTER + ic * P
                    wc = wpool.tile([P, KD, P], f32, tag="w13")
                    nc.sync.dma_start(
                        wc[:],
                        w13_d.ap()[e, :, col:col + P]
                        .rearrange("(ko p) i -> p ko i", p=P))
                    wr = wrpool.tile([P, KD, P], f32r, tag="w13r")
                    nc.gpsimd.tensor_copy(wr[:], wc[:])
                    hp = ph.tile([P, CAP], f32, tag="h2")
                    for k in range(KD):
                        nc.tensor.matmul(hp[:], wr[:, k], xgr[:, k],
                                         start=(k == 0), stop=(k == KD - 1))
                    hps.append(hp)
                bcol = e * 8 + ic
                g1 = g1pool.tile([P, CAP], f32, tag="g1")
                nc.scalar.activation(g1[:], hps[0][:], AF.Silu,
                                     bias=b1s[:, bcol:bcol + 1])
                g3 = g3pool.tile([P, CAP], f32, tag="g3")
                nc.scalar.activation(g3[:], hps[1][:], AF.Identity,
                                     bias=b3s[:, bcol:bcol + 1])
                nc.vector.tensor_mul(hx[:, ic], g1[:], g3[:])

            # FFN layer 2 + transpose-out + per-slot scale
            yT = ytpool.tile([P, NC3, DIM], f32r, tag="yT")
            for d in range(KD):
                yp = ph.tile([P, CAP], f32, tag="h2")
                for i in range(8):
                    w2c = w2pool.tile([P, P], f32, tag="w2")
                    nc.sync.dma_start(
                        w2c[:], w2_d.ap()[e, i * P:(i + 1) * P, d * P:(d + 1) * P])
                    w2r = w2rpool.tile([P, P], f32r, tag="w2r")
                    nc.gpsimd.tensor_copy(w2r[:], w2c[:])
                    nc.tensor.matmul(yp[:], w2r[:], hx[:, i],
                                     start=(i == 0), stop=(i == 7))
                ysb = g3pool.tile([P, CAP], f32, tag="ysb")
                nc.scalar.copy(ysb[:], yp[:])
                for c in range(NC3):
                    tpo = ptr.tile([P, P], f32, tag="tr")
                    nc.tensor.transpose(tpo[:], ysb[:, c * P:(c + 1) * P],
                                        id128[:])
                    nc.scalar.copy(yT[:, c, d * P:(d + 1) * P], tpo[:])

            # combine: acc[b] += Csl_c(:, b)^T @ yT_c
            cwb_b = slbpool.tile([P, TSH], f32, tag="cwb_b")
            nc.sync.dma_start(
                cwb_b[:], cw_dram.ap()[e:e + 1, :].to_broadcast((P, TSH)))
            Csls = []
            for c in range(NC3):
                Csl = cslpool.tile([P, TSH], f32r, tag="Csl", name=f"csl{c}")
                slb = slbpool.tile([P, TSH], f32, tag="slb")
                nc.sync.dma_start(
                    slb[:], slot_dram.ap()[e:e + 1, :].to_broadcast((P, TSH)))
                nc.vector.tensor_scalar(Csl[:], slb[:], ioPf[:],
                                        float(e * CAP + c * P + 1),
                                        op0=OP.subtract, op1=OP.is_equal)
                nc.vector.tensor_mul(Csl[:], Csl[:].bitcast(f32), cwb_b[:])
                Csls.append(Csl)
            for b in range(NB):
                for h in range(2):
                    pp = py.tile([P, 512], f32, tag="y2")
                    for c in range(NC3):
                        nc.tensor.matmul(pp[:], Csls[c][:, b * P:(b + 1) * P],
                                         yT[:, c, h * 512:(h + 1) * 512],
                                         start=(c == 0), stop=(c == NC3 - 1))
                    nc.vector.tensor_add(acc[:, b, h * 512:(h + 1) * 512],
                                         acc[:, b, h * 512:(h + 1) * 512],
                                         pp[:])

        # ---------- shared expert (2 halves x 2 token-halves) ----------
        NQ = 256
        for se in range(E, NE):
            for th in range(4):
                # build feature-major x chunks on the fly (transpose xw_r)
                xsh = xshpool.tile([P, KD, NQ], f32r, tag="xsh")
                for k in range(KD):
                    for q in range(2):
                        b = th * 2 + q
                        tx = ptr.tile([P, P], f32r, tag="tr")
                        nc.tensor.transpose(tx[:],
                                            xw_r[:, b, k * P:(k + 1) * P],
                                            id128r[:])
                        nc.scalar.copy(xsh[:, k, q * P:(q + 1) * P], tx[:])
                hx = hxpool.tile([P, 8, NQ], f32r, tag="hx")
                for ic in range(8):
                    hps = []
                    for w in range(2):
                        col = w * INTER + ic * P
                        wc = wpool.tile([P, KD, P], f32, tag="w13")
                        nc.sync.dma_start(
                            wc[:],
                            w13_d.ap()[se, :, col:col + P]
                            .rearrange("(ko p) i -> p ko i", p=P))
                        wr = wrpool.tile([P, KD, P], f32r, tag="w13r")
                        nc.gpsimd.tensor_copy(wr[:], wc[:])
                        hp = ph.tile([P, NQ], f32, tag="h2")
                        for k in range(KD):
                            nc.tensor.matmul(hp[:], wr[:, k], xsh[:, k],
                                             start=(k == 0), stop=(k == KD - 1))
                        hps.append(hp)
                    bcol = se * 8 + ic
                    g1 = g1pool.tile([P, NQ], f32, tag="g1")
                    nc.scalar.activation(g1[:], hps[0][:], AF.Silu,
                                         bias=b1s[:, bcol:bcol + 1])
                    g3 = g3pool.tile([P, NQ], f32, tag="g3")
                    nc.scalar.activation(g3[:], hps[1][:], AF.Identity,
                                         bias=b3s[:, bcol:bcol + 1])
                    nc.vector.tensor_mul(hx[:, ic], g1[:], g3[:])
                for d in range(KD):
                    yp = ph.tile([P, NQ], f32, tag="h2")
                    for i in range(8):
                        w2c = w2pool.tile([P, P], f32, tag="w2")
                        nc.sync.dma_start(
                            w2c[:],
                            w2_d.ap()[se, i * P:(i + 1) * P, d * P:(d + 1) * P])
                        w2r = w2rpool.tile([P, P], f32r, tag="w2r")
                        nc.gpsimd.tensor_copy(w2r[:], w2c[:])
                        nc.tensor.matmul(yp[:], w2r[:], hx[:, i],
                                         start=(i == 0), stop=(i == 7))
                    zs = g3pool.tile([P, NQ], f32, tag="ysb")
                    if se == E:
                        nc.vector.tensor_scalar(zs[:], yp[:],
                                                sb2s[:, d:d + 1], None,
                                                op0=OP.add)
                    else:
                        nc.vector.tensor_copy(zs[:], yp[:])
                    for q in range(2):
                        b = th * 2 + q
                        tpz = ptr.tile([P, P], f32, tag="tr")
                        nc.tensor.transpose(tpz[:], zs[:, q * P:(q + 1) * P],
                                            id128[:])
                        nc.vector.tensor_add(acc[:, b, d * P:(d + 1) * P],
                                             acc[:, b, d * P:(d + 1) * P],
                                             tpz[:])

        # ---------- output (token-major) ----------
        for b in range(NB):
            nc.sync.dma_start(
                out_d.ap()[b * P:(b + 1) * P, :], acc[:, b])

        for pool in reversed((persist, setup, xtmp, wpool, wrpool, w2pool,
                              w2rpool, misc, g3pool, g1pool, hxpool, cpool,
                              xgpool, ytpool, slbpool, cslpool, xshpool, ph,
                              py, pg, ptr)):
            pool.release()

    nc.compile()
    return nc


def _get_nc():
    import os
    if "nc" not in _CACHE:
        if os.environ.get("KERNEL_V1"):
            _CACHE["nc"] = _build_bass()
            _CACHE["v2"] = False
        else:
            _CACHE["nc"] = _build_bass_v2()
            _CACHE["v2"] = True
    return _CACHE["nc"]


def _prep_shared(gate_w, ew1, eb1, ew2, eb2, ew3, eb3,
                 sw1, sb1, sw2, sb2, sw3, sb3):
    """Host-side packing of (replicated) weight tensors."""
    f = np.float32
    w13 = np.empty((NE, DIM, 2 * INTER), f)
    w2 = np.empty((NE, INTER, DIM), f)
    b1H = np.empty((P, NE * 8), f)
    b3H = np.empty((P, NE * 8), f)
    for e in range(E):
        w13[e, :, :INTER] = ew1[e].T
        w13[e, :, INTER:] = ew3[e].T
        w2[e] = ew2[e].T
        b1H[:, e * 8:(e + 1) * 8] = eb1[e].reshape(8, P).T
        b3H[:, e * 8:(e + 1) * 8] = eb3[e].reshape(8, P).T
    sw1T = sw1.T  # [DIM, 2048]
    sw3T = sw3.T
    sw2T = sw2.T  # [2048, DIM]
    for h in range(2):
        e = E + h
        sl = slice(h * INTER, (h + 1) * INTER)
        w13[e, :, :INTER] = sw1T[:, sl]
        w13[e, :, INTER:] = sw3T[:, sl]
        w2[e] = sw2T[sl, :]
        b1H[:, e * 8:(e + 1) * 8] = sb1[sl].reshape(8, P).T
        b3H[:, e * 8:(e + 1) * 8] = sb3[sl].reshape(8, P).T
    b2e = np.ascontiguousarray(eb2, dtype=f)
    sb2H = np.ascontiguousarray(sb2.reshape(KD, P).T, dtype=f)
    gwT = np.ascontiguousarray(gate_w.T)
    return dict(w13=w13, w2=w2, gwT=gwT, b1H=b1H, b3H=b3H, b2e=b2e, sb2H=sb2H)


def _make_in_maps(inputs):
    shared = _prep_shared(
        inputs["gate_w"], inputs["ew1"], inputs["eb1"], inputs["ew2"],
        inputs["eb2"], inputs["ew3"], inputs["eb3"], inputs["sw1"],
        inputs["sb1"], inputs["sw2"], inputs["sb2"], inputs["sw3"],
        inputs["sb3"])
    x = np.asarray(inputs["x"], np.float32)
    in_maps = []
    for c in range(NCORES):
        m = dict(shared)
        xs = x[c * TSH:(c + 1) * TSH, :]
        m["xT"] = np.ascontiguousarray(xs.T)
        if _CACHE.get("v2"):
            m["xrow"] = np.ascontiguousarray(xs)
        in_maps.append(m)
    return in_maps


def kernel(x, gate_w, ew1, eb1, ew2, eb2, ew3, eb3,
           sw1, sb1, sw2, sb2, sw3, sb3):
    from concourse import bass_utils

    nc = _get_nc()
    in_maps = _make_in_maps(dict(
        x=x, gate_w=gate_w, ew1=ew1, eb1=eb1, ew2=ew2, eb2=eb2, ew3=ew3,
        eb3=eb3, sw1=sw1, sb1=sb1, sw2=sw2, sb2=sb2, sw3=sw3, sb3=sb3))

    res = bass_utils.run_bass_kernel_spmd(
        nc, in_maps, core_ids=list(range(NCORES)), trace=False)

    out = np.empty((T, DIM), np.float32)
    for c in range(NCORES):
        if _CACHE.get("v2"):
            out[c * TSH:(c + 1) * TSH, :] = res.results[c]["outTok"]
        else:
            out[c * TSH:(c + 1) * TSH, :] = res.results[c]["out"].T
    return out


def time_kernel(inputs, iters=5):
    """Dev-only steady-state timing: build the sharded jitted executable once,
    keep inputs device-resident, time repeated executions."""
    import time

    import jax
    import jax.numpy as jnp
    from jax.sharding import Mesh, PartitionSpec
    from jax.experimental.shard_map import shard_map

    import concourse.mybir as mybir
    from concourse import bass2jax

    nc = _get_nc()
    in_maps = _make_in_maps(inputs)

    bass2jax.install_neuronx_cc_hook()

    part_name = nc.partition_id_tensor.name if nc.partition_id_tensor else None
    in_names, out_names, out_avals, zero_outs = [], [], [], []
    for alloc in nc.m.functions[0].allocations:
        if not isinstance(alloc, mybir.MemoryLocationSet):
            continue
        name = alloc.memorylocations[0].name
        if alloc.kind == "ExternalInput":
            if name != part_name:
                in_names.append(name)
        elif alloc.kind == "ExternalOutput":
            out_names.append(name)
            shape = tuple(alloc.tensor_shape)
            dtype = mybir.dt.np(alloc.dtype)
            out_avals.append(jax.core.ShapedArray(shape, dtype))
            zero_outs.append(np.zeros(shape, dtype))
    n_params = len(in_names)
    all_names = in_names + out_names
    if part_name is not None:
        all_names = all_names + [part_name]

    def _body(*args):
        operands = list(args)
        if part_name is not None:
            operands.append(bass2jax.partition_id_tensor())
        outs = bass2jax._bass_exec_p.bind(
            *operands,
            out_avals=tuple(out_avals),
            in_names=tuple(all_names),
            out_names=tuple(out_names),
            lowering_input_output_aliases=(),
            sim_require_finite=True,
            sim_require_nnan=True,
            nc=nc,
        )
        return tuple(outs)

    devices = jax.devices()[:NCORES]
    mesh = Mesh(np.asarray(devices), ("core",))
    in_specs = (PartitionSpec("core"),) * (n_params + len(out_names))
    out_specs = (PartitionSpec("core"),) * len(out_names)
    sharded = jax.jit(
        shard_map(_body, mesh=mesh, in_specs=in_specs, out_specs=out_specs,
                  check_rep=False),
        keep_unused=True,
    )
    concat_in = [
        np.concatenate([np.asarray(in_maps[c][n]) for c in range(NCORES)], axis=0)
        for n in in_names
    ]
    concat_zeros = [
        np.zeros((NCORES * z.shape[0], *z.shape[1:]), z.dtype) for z in zero_outs
    ]
    sharding = jax.sharding.NamedSharding(mesh, PartitionSpec("core"))
    dev_in = [jax.device_put(a, sharding) for a in concat_in]
    dev_zero = [jax.device_put(a, sharding) for a in concat_zeros]

    times = []
    out = sharded(*dev_in, *dev_zero)   # warmup/compile
    jax.block_until_ready(out)
    for _ in range(iters):
        t0 = time.perf_counter()
        out = sharded(*dev_in, *dev_zero)
        jax.block_until_ready(out)
        times.append(time.perf_counter() - t0)
    return times


def time_kernel_chained(inputs, chain=8, iters=3):
    """Chain `chain` kernel executions inside one jitted call, feeding the
    output back as xT.  Per-kernel time = slope between chain lengths."""
    import time

    import jax
    import jax.numpy as jnp
    from jax.sharding import Mesh, PartitionSpec
    from jax.experimental.shard_map import shard_map

    import concourse.mybir as mybir
    from concourse import bass2jax

    nc = _get_nc()
    in_maps = _make_in_maps(inputs)
    bass2jax.install_neuronx_cc_hook()

    part_name = nc.partition_id_tensor.name if nc.partition_id_tensor else None
    in_names, out_names, out_avals = [], [], []
    for alloc in nc.m.functions[0].allocations:
        if not isinstance(alloc, mybir.MemoryLocationSet):
            continue
        name = alloc.memorylocations[0].name
        if alloc.kind == "ExternalInput":
            if name != part_name:
                in_names.append(name)
        elif alloc.kind == "ExternalOutput":
            out_names.append(name)
            out_avals.append(jax.core.ShapedArray(
                tuple(alloc.tensor_shape), mybir.dt.np(alloc.dtype)))
    all_names = in_names + out_names
    if part_name is not None:
        all_names = all_names + [part_name]
    xt_pos = in_names.index("xT")

    def _one(args_by_name, zero_buf):
        operands = [args_by_name[n] for n in in_names]
        operands.append(zero_buf)
        if part_name is not None:
            operands.append(bass2jax.partition_id_tensor())
        outs = bass2jax._bass_exec_p.bind(
            *operands,
            out_avals=tuple(out_avals),
            in_names=tuple(all_names),
            out_names=tuple(out_names),
            lowering_input_output_aliases=(),
            sim_require_finite=True,
            sim_require_nnan=True,
            nc=nc,
        )
        return outs[0]

    def _chain_body(*args):
        d = dict(zip(in_names, args[:len(in_names)]))
        zeros = args[len(in_names):]
        out = _one(d, zeros[0])
        for j in range(chain - 1):
            d["xT"] = out
            out = _one(d, zeros[j + 1])
        return (out,)

    devices = jax.devices()[:NCORES]
    mesh = Mesh(np.asarray(devices), ("core",))
    in_specs = (PartitionSpec("core"),) * (len(in_names) + chain)
    out_specs = (PartitionSpec("core"),)
    sharded = jax.jit(
        shard_map(_chain_body, mesh=mesh, in_specs=in_specs,
                  out_specs=out_specs, check_rep=False),
        keep_unused=True,
    )
    concat_in = [
        np.concatenate([np.asarray(in_maps[c][n]) for c in range(NCORES)], axis=0)
        for n in in_names
    ]
    zshape = (NCORES * out_avals[0].shape[0], *out_avals[0].shape[1:])
    concat_in += [np.zeros(zshape, np.float32) for _ in range(chain)]
    sharding = jax.sharding.NamedSharding(mesh, PartitionSpec("core"))
    dev_in = [jax.device_put(a, sharding) for a in concat_in]

    out = sharded(*dev_in)
    jax.block_until_ready(out)
    times = []
    for _ in range(iters):
        t0 = time.perf_counter()
        out = sharded(*dev_in)
        jax.block_until_ready(out)
        times.append(time.perf_counter() - t0)
    return times


# revision 20
# speedup vs baseline: 1.1685x; 1.1685x over previous
"""Trainium2 Bass kernel for MoE (nn_MoE_75170517615144).

Strategy: data-parallel over tokens (1024 tokens/core x 8 cores), all 16
routed experts + shared expert computed per core on its token shard.
Everything runs feature-major ([feature, token] layout) so the FFN needs no
transposes; matmuls use float32r (full-rate fp32 on the PE).

v1 = masked-dense: every expert computed for every token, combine weights
zero out unselected experts (matches reference semantics directly).

Host side: shards x, pre-transposes weights, feeds 8 identical-program cores
via run_bass_kernel_spmd; output is concatenated + transposed back.
"""
import sys

sys.path.insert(0, "/opt/trn_rl_repo")

import numpy as np

DIM = 1024
INTER = 1024
E = 16          # routed experts
NE = 18         # + shared expert split into 2 pseudo-experts
SINTER = 2048
T = 8192
NCORES = 8
TSH = T // NCORES   # tokens per core
P = 128
KD = DIM // P       # 8 contraction chunks

_CACHE = {}


def _build_bass():
    import concourse.bacc as bacc
    import concourse.tile as tile
    import concourse.mybir as mybir
    from concourse.masks import make_identity

    f32 = mybir.dt.float32
    f32r = mybir.dt.float32r
    AF = mybir.ActivationFunctionType
    OP = mybir.AluOpType
    AX = mybir.AxisListType

    nc = bacc.Bacc("TRN2", target_bir_lowering=False, debug=False)

    xT_d = nc.dram_tensor("xT", [DIM, TSH], f32, kind="ExternalInput")
    w13_d = nc.dram_tensor("w13", [NE, DIM, 2 * INTER], f32, kind="ExternalInput")
    w2_d = nc.dram_tensor("w2", [NE, INTER, DIM], f32, kind="ExternalInput")
    gwT_d = nc.dram_tensor("gwT", [DIM, E], f32, kind="ExternalInput")
    b1_d = nc.dram_tensor("b1H", [P, NE * 8], f32, kind="ExternalInput")
    b3_d = nc.dram_tensor("b3H", [P, NE * 8], f32, kind="ExternalInput")
    b2_d = nc.dram_tensor("b2e", [16, DIM], f32, kind="ExternalInput")
    sb2_d = nc.dram_tensor("sb2H", [P, KD], f32, kind="ExternalInput")
    out_d = nc.dram_tensor("out", [DIM, TSH], f32, kind="ExternalOutput")
    cw_dram = nc.dram_tensor("cw_scratch", [E, TSH], f32)  # internal scratch

    with tile.TileContext(nc) as tc:
        persist = tc.alloc_tile_pool(name="persist", bufs=1)
        setup = tc.alloc_tile_pool(name="setup", bufs=1)
        xtmp = tc.alloc_tile_pool(name="xtmp", bufs=2)
        wpool = tc.alloc_tile_pool(name="wpool", bufs=2)
        wrpool = tc.alloc_tile_pool(name="wrpool", bufs=2)
        w2pool = tc.alloc_tile_pool(name="w2pool", bufs=4)
        w2rpool = tc.alloc_tile_pool(name="w2rpool", bufs=4)
        misc = tc.alloc_tile_pool(name="misc", bufs=2)
        g3pool = tc.alloc_tile_pool(name="g3pool", bufs=2)
        g1pool = tc.alloc_tile_pool(name="g1pool", bufs=2)
        hxpool = tc.alloc_tile_pool(name="hxpool", bufs=1)
        cwepool = tc.alloc_tile_pool(name="cwepool", bufs=2)
        ph = tc.alloc_tile_pool(name="ph", bufs=2, space="PSUM")
        py = tc.alloc_tile_pool(name="py", bufs=2, space="PSUM")

        # ---------- load x (transposed), round to f32r, and gate ----------
        # The gate matmul runs in plain fp32 (exact) off the staging tiles so
        # the top-4 selection tie-breaks identically to the fp32 reference.
        gw = setup.tile([P, KD, E], f32, tag="gw")
        nc.sync.dma_start(gw[:], gwT_d.ap().rearrange("(ko p) e -> p ko e", p=P))

        xr = persist.tile([P, KD, TSH], f32r, tag="xr")
        lg_ps = ph.tile([E, TSH], f32, tag="h")
        for k in range(KD):
            xt = xtmp.tile([P, TSH], f32, tag="xt")
            nc.sync.dma_start(xt[:], xT_d.ap()[k * P:(k + 1) * P, :])
            nc.vector.tensor_copy(xr[:, k], xt[:])
            nc.tensor.matmul(lg_ps[:, :512], gw[:, k], xt[:, :512],
                             start=(k == 0), stop=(k == KD - 1))
            nc.tensor.matmul(lg_ps[:, 512:], gw[:, k], xt[:, 512:],
                             start=(k == 0), stop=(k == KD - 1))
        lg = setup.tile([E, TSH], f32, tag="lg")
        nc.vector.tensor_copy(lg[:], lg_ps[:])

        id16 = persist.tile([16, 16], f32, tag="id16")
        make_identity(nc, id16[:])
        id128 = persist.tile([P, P], f32, tag="id128")
        make_identity(nc, id128[:])

        # cw16r: combine weights, expert-major.  rhs of the bias-combine
        # matmul; also written to DRAM for the per-expert broadcast loads.
        cw16r = persist.tile([16, TSH], f32r, tag="cw16r")

        nblk = TSH // P
        for b in range(nblk):
            ltp = py.tile([P, 16], f32, tag="y")
            nc.tensor.transpose(ltp[:], lg[:, b * P:(b + 1) * P], id16[:])
            lt = misc.tile([P, 16], f32, tag="lt")
            nc.vector.tensor_copy(lt[:], ltp[:])
            # softmax over the 16 experts (free axis)
            nm = misc.tile([P, 1], f32, tag="nm")
            nc.vector.tensor_reduce(nm[:], lt[:], axis=AX.X, op=OP.max, negate=True)
            es = misc.tile([P, 16], f32, tag="es")
            nc.scalar.activation(es[:], lt[:], AF.Exp, bias=nm[:])
            sm = misc.tile([P, 1], f32, tag="sm")
            nc.vector.tensor_reduce(sm[:], es[:], axis=AX.X, op=OP.add)
            rs = misc.tile([P, 1], f32, tag="rs")
            nc.vector.reciprocal(rs[:], sm[:])
            sS = misc.tile([P, 16], f32, tag="sS")
            nc.vector.tensor_scalar_mul(sS[:], es[:], rs[:])
            # top-4 mask
            t8 = misc.tile([P, 8], f32, tag="t8")
            nc.vector.max(t8[:], sS[:])
            mk = misc.tile([P, 16], f32, tag="mk")
            nc.vector.tensor_scalar(mk[:], sS[:], t8[:, 3:4], None, op0=OP.is_ge)
            cwb = misc.tile([P, 16], f32, tag="cwb")
            nc.vector.tensor_mul(cwb[:], sS[:], mk[:])
            # transpose back to expert-major
            ctp = py.tile([16, P], f32, tag="y")
            nc.tensor.transpose(ctp[:], cwb[:], id128[:])
            nc.vector.tensor_copy(cw16r[:, b * P:(b + 1) * P], ctp[:])

        # stash cw rows in DRAM for cheap partition-broadcast reads
        nc.sync.dma_start(cw_dram.ap(), cw16r[:].bitcast(f32))

        # ---------- bias-combine init of y accumulator ----------
        b2s = setup.tile([16, DIM], f32, tag="b2s")
        nc.sync.dma_start(b2s[:], b2_d.ap())
        b2r = setup.tile([16, DIM], f32r, tag="b2r")
        nc.vector.tensor_copy(b2r[:], b2s[:])
        sb2s = persist.tile([P, KD], f32, tag="sb2s")
        nc.sync.dma_start(sb2s[:], sb2_d.ap())

        y_acc = persist.tile([P, KD, TSH], f32, tag="yacc")
        for d in range(KD):
            yp = py.tile([P, TSH], f32, tag="y")
            nc.tensor.matmul(yp[:, :512], b2r[:, d * P:(d + 1) * P],
                             cw16r[:, :512], start=True, stop=True)
            nc.tensor.matmul(yp[:, 512:], b2r[:, d * P:(d + 1) * P],
                             cw16r[:, 512:], start=True, stop=True)
            nc.vector.tensor_copy(y_acc[:, d], yp[:])

        # ---------- biases ----------
        b1s = persist.tile([P, NE * 8], f32, tag="b1s")
        nc.sync.dma_start(b1s[:], b1_d.ap())
        b3s = persist.tile([P, NE * 8], f32, tag="b3s")
        nc.sync.dma_start(b3s[:], b3_d.ap())

        # ---------- expert loop (16 routed + 2 shared halves) ----------
        for e in range(NE):
            routed = e < E
            if routed:
                cwe = cwepool.tile([P, TSH], f32, tag="cwe")
                nc.sync.dma_start(
                    cwe[:], cw_dram.ap()[e:e + 1, :].to_broadcast((P, TSH)))

            hx = hxpool.tile([P, 8, TSH], f32r, tag="hx")
            for ic in range(8):
                # stream w1 and w3 column-blocks for this i-chunk
                hps = []
                for w in range(2):
                    col = w * INTER + ic * P
                    wc = wpool.tile([P, KD, P], f32, tag="w13")
                    nc.sync.dma_start(
                        wc[:],
                        w13_d.ap()[e, :, col:col + P]
                        .rearrange("(ko p) i -> p ko i", p=P))
                    wr = wrpool.tile([P, KD, P], f32r, tag="w13r")
                    nc.gpsimd.tensor_copy(wr[:], wc[:])
                    hp = ph.tile([P, TSH], f32, tag="h")
                    for k in range(KD):
                        nc.tensor.matmul(hp[:, :512], wr[:, k], xr[:, k, :512],
                                         start=(k == 0), stop=(k == KD - 1))
                        nc.tensor.matmul(hp[:, 512:], wr[:, k], xr[:, k, 512:],
                                         start=(k == 0), stop=(k == KD - 1))
                    hps.append(hp)
                bcol = e * 8 + ic
                g1 = g1pool.tile([P, TSH], f32, tag="g1")
                nc.scalar.activation(g1[:], hps[0][:], AF.Silu,
                                     bias=b1s[:, bcol:bcol + 1])
                g3 = g3pool.tile([P, TSH], f32, tag="g3")
                nc.scalar.activation(g3[:], hps[1][:], AF.Identity,
                                     bias=b3s[:, bcol:bcol + 1])
                nc.vector.tensor_mul(hx[:, ic], g1[:], g3[:])
                if routed:
                    nc.vector.tensor_mul(hx[:, ic], hx[:, ic], cwe[:])

            for d in range(KD):
                yp = py.tile([P, TSH], f32, tag="y")
                for i in range(8):
                    w2c = w2pool.tile([P, P], f32, tag="w2")
                    nc.sync.dma_start(
                        w2c[:], w2_d.ap()[e, i * P:(i + 1) * P, d * P:(d + 1) * P])
                    w2r = w2rpool.tile([P, P], f32r, tag="w2r")
                    nc.gpsimd.tensor_copy(w2r[:], w2c[:])
                    nc.tensor.matmul(yp[:, :512], w2r[:], hx[:, i, :512],
                                     start=(i == 0), stop=(i == 7))
                    nc.tensor.matmul(yp[:, 512:], w2r[:], hx[:, i, 512:],
                                     start=(i == 0), stop=(i == 7))
                nc.vector.tensor_add(y_acc[:, d], y_acc[:, d], yp[:])

        # ---------- output (+ shared-expert output bias sb2) ----------
        for d in range(KD):
            nc.vector.tensor_scalar(y_acc[:, d], y_acc[:, d],
                                    sb2s[:, d:d + 1], None, op0=OP.add)
            nc.sync.dma_start(out_d.ap()[d * P:(d + 1) * P, :], y_acc[:, d])

        for pool in reversed((persist, setup, xtmp, wpool, wrpool, w2pool,
                              w2rpool, misc, g3pool, g1pool, hxpool, cwepool,
                              ph, py)):
            pool.release()

    nc.compile()
    return nc




CAP = 384           # padded dispatch capacity per expert (mean 256, 9 sigma)
NC3 = CAP // 128
NB = TSH // P       # token tiles per core


def _build_bass_v2():
    """Sparse-dispatch variant: top-4 routing drives one-hot dispatch
    matrices built on the DVE; gather, per-slot gating, and combine all run
    as PE matmuls.  Routed experts compute on CAP=384 gathered tokens
    instead of all 1024."""
    import concourse.bacc as bacc
    import concourse.tile as tile
    import concourse.mybir as mybir
    from concourse.masks import make_identity

    f32 = mybir.dt.float32
    f32r = mybir.dt.float32r
    i32 = mybir.dt.int32
    AF = mybir.ActivationFunctionType
    OP = mybir.AluOpType
    AX = mybir.AxisListType

    nc = bacc.Bacc("TRN2", target_bir_lowering=False, debug=False)

    xT_d = nc.dram_tensor("xT", [DIM, TSH], f32, kind="ExternalInput")
    xrow_d = nc.dram_tensor("xrow", [TSH, DIM], f32, kind="ExternalInput")
    w13_d = nc.dram_tensor("w13", [NE, DIM, 2 * INTER], f32, kind="ExternalInput")
    w2_d = nc.dram_tensor("w2", [NE, INTER, DIM], f32, kind="ExternalInput")
    gwT_d = nc.dram_tensor("gwT", [DIM, E], f32, kind="ExternalInput")
    b1_d = nc.dram_tensor("b1H", [P, NE * 8], f32, kind="ExternalInput")
    b3_d = nc.dram_tensor("b3H", [P, NE * 8], f32, kind="ExternalInput")
    b2_d = nc.dram_tensor("b2e", [16, DIM], f32, kind="ExternalInput")
    sb2_d = nc.dram_tensor("sb2H", [P, KD], f32, kind="ExternalInput")
    out_d = nc.dram_tensor("outTok", [TSH, DIM], f32, kind="ExternalOutput")
    slot_dram = nc.dram_tensor("slot_scratch", [E, TSH], f32)
    cw_dram = nc.dram_tensor("cw_scratch2", [E, TSH], f32)

    with tile.TileContext(nc) as tc:
        persist = tc.alloc_tile_pool(name="persist", bufs=1)
        setup = tc.alloc_tile_pool(name="setup", bufs=1)
        xtmp = tc.alloc_tile_pool(name="xtmp", bufs=1)
        wpool = tc.alloc_tile_pool(name="wpool", bufs=2)
        wrpool = tc.alloc_tile_pool(name="wrpool", bufs=2)
        w2pool = tc.alloc_tile_pool(name="w2pool", bufs=4)
        w2rpool = tc.alloc_tile_pool(name="w2rpool", bufs=4)
        misc = tc.alloc_tile_pool(name="misc", bufs=2)
        g3pool = tc.alloc_tile_pool(name="g3pool", bufs=2)
        g1pool = tc.alloc_tile_pool(name="g1pool", bufs=2)
        hxpool = tc.alloc_tile_pool(name="hxpool", bufs=1)
        cpool = tc.alloc_tile_pool(name="cpool", bufs=1)
        xgpool = tc.alloc_tile_pool(name="xgpool", bufs=1)
        ytpool = tc.alloc_tile_pool(name="ytpool", bufs=1)
        slbpool = tc.alloc_tile_pool(name="slbpool", bufs=1)
        cslpool = tc.alloc_tile_pool(name="cslpool", bufs=3)
        xshpool = tc.alloc_tile_pool(name="xshpool", bufs=1)
        ph = tc.alloc_tile_pool(name="ph", bufs=2, space="PSUM")
        py = tc.alloc_tile_pool(name="py", bufs=2, space="PSUM")
        pg = tc.alloc_tile_pool(name="pg", bufs=2, space="PSUM")
        ptr = tc.alloc_tile_pool(name="ptr", bufs=2, space="PSUM")

        # ---------- load x row-major (rounded) + gate logits (exact fp32) ---
        gw = setup.tile([P, KD, E], f32, tag="gw")
        nc.sync.dma_start(gw[:], gwT_d.ap().rearrange("(ko p) e -> p ko e", p=P))

        xw_r = persist.tile([P, NB, DIM], f32r, tag="xwr")
        for b in range(NB):
            xt = xtmp.tile([P, DIM], f32, tag="xt")
            nc.sync.dma_start(xt[:], xrow_d.ap()[b * P:(b + 1) * P, :])
            nc.vector.tensor_copy(xw_r[:, b], xt[:])

        lg = setup.tile([E, TSH], f32, tag="lg")
        lg_hs = [ph.tile([E, 512], f32, tag="h2", name=f"lg_h{_h}")
                 for _h in range(2)]
        for k in range(KD):
            xtc = xtmp.tile([P, TSH], f32, tag="xt")
            nc.sync.dma_start(xtc[:], xT_d.ap()[k * P:(k + 1) * P, :])
            for h in range(2):
                nc.tensor.matmul(lg_hs[h][:], gw[:, k],
                                 xtc[:, h * 512:(h + 1) * 512],
                                 start=(k == 0), stop=(k == KD - 1))
        for h in range(2):
            nc.vector.tensor_copy(lg[:, h * 512:(h + 1) * 512], lg_hs[h][:])

        id16 = persist.tile([16, 16], f32, tag="id16")
        make_identity(nc, id16[:])
        id128 = persist.tile([P, P], f32, tag="id128")
        make_identity(nc, id128[:])
        id128r = persist.tile([P, P], f32r, tag="id128r")
        nc.vector.tensor_copy(id128r[:], id128[:])

        cw16r = persist.tile([16, TSH], f32r, tag="cw16r")
        cwTok = persist.tile([P, NB, 16], f32, tag="cwTok")

        for b in range(NB):
            ltp = ptr.tile([P, 16], f32, tag="tr")
            nc.tensor.transpose(ltp[:], lg[:, b * P:(b + 1) * P], id16[:])
            lt = misc.tile([P, 16], f32, tag="lt")
            nc.vector.tensor_copy(lt[:], ltp[:])
            nm = misc.tile([P, 1], f32, tag="nm")
            nc.vector.tensor_reduce(nm[:], lt[:], axis=AX.X, op=OP.max, negate=True)
            es = misc.tile([P, 16], f32, tag="es")
            nc.scalar.activation(es[:], lt[:], AF.Exp, bias=nm[:])
            sm = misc.tile([P, 1], f32, tag="sm")
            nc.vector.tensor_reduce(sm[:], es[:], axis=AX.X, op=OP.add)
            rs = misc.tile([P, 1], f32, tag="rs")
            nc.vector.reciprocal(rs[:], sm[:])
            sS = misc.tile([P, 16], f32, tag="sS")
            nc.vector.tensor_scalar_mul(sS[:], es[:], rs[:])
            t8 = misc.tile([P, 8], f32, tag="t8")
            nc.vector.max(t8[:], sS[:])
            mk = misc.tile([P, 16], f32, tag="mk")
            nc.vector.tensor_scalar(mk[:], sS[:], t8[:, 3:4], None, op0=OP.is_ge)
            nc.vector.tensor_mul(cwTok[:, b], sS[:], mk[:])
            ctp = ptr.tile([16, P], f32, tag="tr")
            nc.tensor.transpose(ctp[:], cwTok[:, b], id128[:])
            nc.vector.tensor_copy(cw16r[:, b * P:(b + 1) * P], ctp[:])

        # ---------- slot machinery (expert-major) ----------
        cwf = cw16r[:].bitcast(f32)
        maskT = setup.tile([16, TSH], f32, tag="maskT")
        nc.vector.tensor_scalar(maskT[:], cwf, 0.0, None, op0=OP.is_gt)
        incl = setup.tile([16, TSH], f32, tag="incl")
        nc.vector.tensor_tensor_scan(incl[:], maskT[:], maskT[:], 0.0,
                                     op0=OP.add, op1=OP.bypass)
        io16 = setup.tile([16, 1], i32, tag="io16")
        nc.gpsimd.iota(io16[:], pattern=[[0, 1]], base=1, channel_multiplier=CAP)
        io16f = setup.tile([16, 1], f32, tag="io16f")
        nc.vector.tensor_copy(io16f[:], io16[:])
        nc.vector.tensor_sub(incl[:], incl[:], maskT[:])
        nc.vector.tensor_scalar(incl[:], incl[:], io16f[:], None, op0=OP.add)
        sl1 = incl
        nc.vector.tensor_mul(sl1[:], sl1[:], maskT[:])   # slot+1 or 0
        nc.sync.dma_start(slot_dram.ap(), sl1[:])
        nc.sync.dma_start(cw_dram.ap(), cw16r[:].bitcast(f32))

        # slot+1 token-major [128, NB, 16]
        slotTok = persist.tile([P, NB, 16], f32, tag="slotTok")
        for b in range(NB):
            stp = ptr.tile([P, 16], f32, tag="tr")
            nc.tensor.transpose(stp[:], sl1[:, b * P:(b + 1) * P], id16[:])
            nc.vector.tensor_copy(slotTok[:, b], stp[:])

        ioP = persist.tile([P, 1], i32, tag="ioP")
        nc.gpsimd.iota(ioP[:], pattern=[[0, 1]], base=0, channel_multiplier=1)
        ioPf = persist.tile([P, 1], f32, tag="ioPf")
        nc.vector.tensor_copy(ioPf[:], ioP[:])
        ioJ = persist.tile([P, CAP], i32, tag="ioJ")
        nc.gpsimd.iota(ioJ[:], pattern=[[1, CAP]], base=1, channel_multiplier=0)
        ioJf = persist.tile([P, CAP], f32, tag="ioJf")
        nc.vector.tensor_copy(ioJf[:], ioJ[:])

        # ---------- biases ----------
        b2s = setup.tile([16, DIM], f32, tag="b2s")
        nc.sync.dma_start(b2s[:], b2_d.ap())
        b2r = setup.tile([16, DIM], f32r, tag="b2r")
        nc.vector.tensor_copy(b2r[:], b2s[:])
        sb2s = persist.tile([P, KD], f32, tag="sb2s")
        nc.sync.dma_start(sb2s[:], sb2_d.ap())
        b1s = persist.tile([P, NE * 8], f32, tag="b1s")
        nc.sync.dma_start(b1s[:], b1_d.ap())
        b3s = persist.tile([P, NE * 8], f32, tag="b3s")
        nc.sync.dma_start(b3s[:], b3_d.ap())

        # ---------- bias-combine init of token-major accumulator ----------
        acc = persist.tile([P, NB, DIM], f32, tag="acc")
        for b in range(NB):
            for h in range(2):
                ap_ = py.tile([P, 512], f32, tag="y2")
                nc.tensor.matmul(ap_[:], cw16r[:, b * P:(b + 1) * P],
                                 b2r[:, h * 512:(h + 1) * 512],
                                 start=True, stop=True)
                nc.vector.tensor_copy(acc[:, b, h * 512:(h + 1) * 512], ap_[:])

        # ---------- routed experts ----------
        for e in range(E):
            # one-hot dispatch (token-major): CeT[t, j] = (j+e*CAP+1 == slot+1)
            CeT = cpool.tile([P, NB, CAP], f32r, tag="CeT")
            for b in range(NB):
                nc.vector.tensor_scalar(CeT[:, b], ioJf[:],
                                        slotTok[:, b, e:e + 1],
                                        float(-e * CAP),
                                        op0=OP.subtract, op1=OP.is_equal)

            # gather: xgr[d_chunk] = sum_b xrow_b(d_chunk)^T @ CeT_b
            xgr = xgpool.tile([P, KD, CAP], f32r, tag="xgr")
            for d in range(KD):
                gp = pg.tile([P, CAP], f32, tag="g")
                for b in range(NB):
                    nc.tensor.matmul(gp[:], xw_r[:, b, d * P:(d + 1) * P],
                                     CeT[:, b], start=(b == 0),
                                     stop=(b == NB - 1))
                nc.scalar.copy(xgr[:, d], gp[:])

            # FFN layer 1 on gathered tokens
            hx = hxpool.tile([P, 8, CAP], f32r, tag="hx")
            for ic in range(8):
                hps = []
                for w in range(2):
                    col = w * INTER + ic * P
                    wc = wpool.tile([P, KD, P], f32, tag="w13")
                    nc.sync.dma_start(
                        wc[:],
                        w13_d.ap()[e, :, col:col + P]
                        .rearrange("(ko p) i -> p ko i", p=P))
                    wr = wrpool.tile([P, KD, P], f32r, tag="w13r")
                    nc.gpsimd.tensor_copy(wr[:], wc[:])
                    hp = ph.tile([P, CAP], f32, tag="h2")
                    for k in range(KD):
                        nc.tensor.matmul(hp[:], wr[:, k], xgr[:, k],
                                         start=(k == 0), stop=(k == KD - 1))
                    hps.append(hp)
                bcol = e * 8 + ic
                g1 = g1pool.tile([P, CAP], f32, tag="g1")
                nc.scalar.activation(g1[:], hps[0][:], AF.Silu,
                                     bias=b1s[:, bcol:bcol + 1])
                g3 = g3pool.tile([P, CAP], f32, tag="g3")
                nc.scalar.activation(g3[:], hps[1][:], AF.Identity,
                                     bias=b3s[:, bcol:bcol + 1])
                nc.vector.tensor_mul(hx[:, ic], g1[:], g3[:])

            # FFN layer 2 + transpose-out + per-slot scale
            yT = ytpool.tile([P, NC3, DIM], f32r, tag="yT")
            for d in range(KD):
                yp = ph.tile([P, CAP], f32, tag="h2")
                for i in range(8):
                    w2c = w2pool.tile([P, P], f32, tag="w2")
                    nc.sync.dma_start(
                        w2c[:], w2_d.ap()[e, i * P:(i + 1) * P, d * P:(d + 1) * P])
                    w2r = w2rpool.tile([P, P], f32r, tag="w2r")
                    nc.gpsimd.tensor_copy(w2r[:], w2c[:])
                    nc.tensor.matmul(yp[:], w2r[:], hx[:, i],
                                     start=(i == 0), stop=(i == 7))
                ysb = g3pool.tile([P, CAP], f32, tag="ysb")
                nc.scalar.copy(ysb[:], yp[:])
                for c in range(NC3):
                    tpo = ptr.tile([P, P], f32, tag="tr")
                    nc.tensor.transpose(tpo[:], ysb[:, c * P:(c + 1) * P],
                                        id128[:])
                    nc.scalar.copy(yT[:, c, d * P:(d + 1) * P], tpo[:])

            # combine: acc[b] += Csl_c(:, b)^T @ yT_c
            cwb_b = slbpool.tile([P, TSH], f32, tag="cwb_b")
            nc.sync.dma_start(
                cwb_b[:], cw_dram.ap()[e:e + 1, :].to_broadcast((P, TSH)))
            Csls = []
            for c in range(NC3):
                Csl = cslpool.tile([P, TSH], f32r, tag="Csl", name=f"csl{c}")
                slb = slbpool.tile([P, TSH], f32, tag="slb")
                nc.sync.dma_start(
                    slb[:], slot_dram.ap()[e:e + 1, :].to_broadcast((P, TSH)))
                nc.vector.tensor_scalar(Csl[:], slb[:], ioPf[:],
                                        float(e * CAP + c * P + 1),
                                        op0=OP.subtract, op1=OP.is_equal)
                nc.vector.tensor_mul(Csl[:], Csl[:].bitcast(f32), cwb_b[:])
                Csls.append(Csl)
            for b in range(NB):
                for h in range(2):
                    pp = py.tile([P, 512], f32, tag="y2")
                    for c in range(NC3):
                        nc.tensor.matmul(pp[:], Csls[c][:, b * P:(b + 1) * P],
                                         yT[:, c, h * 512:(h + 1) * 512],
                                         start=(c == 0), stop=(c == NC3 - 1))
                    nc.vector.tensor_add(acc[:, b, h * 512:(h + 1) * 512],
                                         acc[:, b, h * 512:(h + 1) * 512],
                                         pp[:])

        # ---------- shared expert (2 halves x 2 token-halves) ----------
        NQ = 256
        for se in range(E, NE):
            for th in range(4):
                # build feature-major x chunks on the fly (transpose xw_r)
                xsh = xshpool.tile([P, KD, NQ], f32r, tag="xsh")
                for k in range(KD):
                    for q in range(2):
                        b = th * 2 + q
                        tx = ptr.tile([P, P], f32r, tag="tr")
                        nc.tensor.transpose(tx[:],
                                            xw_r[:, b, k * P:(k + 1) * P],
                                            id128r[:])
                        nc.scalar.copy(xsh[:, k, q * P:(q + 1) * P], tx[:])
                hx = hxpool.tile([P, 8, NQ], f32r, tag="hx")
                for ic in range(8):
                    hps = []
                    for w in range(2):
                        col = w * INTER + ic * P
                        wc = wpool.tile([P, KD, P], f32, tag="w13")
                        nc.sync.dma_start(
                            wc[:],
                            w13_d.ap()[se, :, col:col + P]
                            .rearrange("(ko p) i -> p ko i", p=P))
                        wr = wrpool.tile([P, KD, P], f32r, tag="w13r")
                        nc.gpsimd.tensor_copy(wr[:], wc[:])
                        hp = ph.tile([P, NQ], f32, tag="h2")
                        for k in range(KD):
                            nc.tensor.matmul(hp[:], wr[:, k], xsh[:, k],
                                             start=(k == 0), stop=(k == KD - 1))
                        hps.append(hp)
                    bcol = se * 8 + ic
                    g1 = g1pool.tile([P, NQ], f32, tag="g1")
                    nc.scalar.activation(g1[:], hps[0][:], AF.Silu,
                                         bias=b1s[:, bcol:bcol + 1])
                    g3 = g3pool.tile([P, NQ], f32, tag="g3")
                    nc.scalar.activation(g3[:], hps[1][:], AF.Identity,
                                         bias=b3s[:, bcol:bcol + 1])
                    nc.vector.tensor_mul(hx[:, ic], g1[:], g3[:])
                for d in range(KD):
                    yp = ph.tile([P, NQ], f32, tag="h2")
                    for i in range(8):
                        w2c = w2pool.tile([P, P], f32, tag="w2")
                        nc.sync.dma_start(
                            w2c[:],
                            w2_d.ap()[se, i * P:(i + 1) * P, d * P:(d + 1) * P])
                        w2r = w2rpool.tile([P, P], f32r, tag="w2r")
                        nc.gpsimd.tensor_copy(w2r[:], w2c[:])
                        nc.tensor.matmul(yp[:], w2r[:], hx[:, i],
                                         start=(i == 0), stop=(i == 7))
                    zs = g3pool.tile([P, NQ], f32, tag="ysb")
                    if se == E:
                        nc.vector.tensor_scalar(zs[:], yp[:],
                                                sb2s[:, d:d + 1], None,
                                                op0=OP.add)
                    else:
                        nc.vector.tensor_copy(zs[:], yp[:])
                    for q in range(2):
                        b = th * 2 + q
                        tpz = ptr.tile([P, P], f32, tag="tr")
                        nc.tensor.transpose(tpz[:], zs[:, q * P:(q + 1) * P],
                                            id128[:])
                        nc.vector.tensor_add(acc[:, b, d * P:(d + 1) * P],
                                             acc[:, b, d * P:(d + 1) * P],
                                             tpz[:])

        # ---------- output (token-major) ----------
        for b in range(NB):
            nc.sync.dma_start(
                out_d.ap()[b * P:(b + 1) * P, :], acc[:, b])

        for pool in reversed((persist, setup, xtmp, wpool, wrpool, w2pool,
                              w2rpool, misc, g3pool, g1pool, hxpool, cpool,
                              xgpool, ytpool, slbpool, cslpool, xshpool, ph,
                              py, pg, ptr)):
            pool.release()

    nc.compile()
    return nc


def _get_nc():
    import os
    if "nc" not in _CACHE:
        if os.environ.get("KERNEL_V2"):
            _CACHE["nc"] = _build_bass_v2()
            _CACHE["v2"] = True
        else:
            _CACHE["nc"] = _build_bass()
            _CACHE["v2"] = False
    return _CACHE["nc"]


def _prep_shared(gate_w, ew1, eb1, ew2, eb2, ew3, eb3,
                 sw1, sb1, sw2, sb2, sw3, sb3):
    """Host-side packing of (replicated) weight tensors."""
    f = np.float32
    w13 = np.empty((NE, DIM, 2 * INTER), f)
    w2 = np.empty((NE, INTER, DIM), f)
    b1H = np.empty((P, NE * 8), f)
    b3H = np.empty((P, NE * 8), f)
    for e in range(E):
        w13[e, :, :INTER] = ew1[e].T
        w13[e, :, INTER:] = ew3[e].T
        w2[e] = ew2[e].T
        b1H[:, e * 8:(e + 1) * 8] = eb1[e].reshape(8, P).T
        b3H[:, e * 8:(e + 1) * 8] = eb3[e].reshape(8, P).T
    sw1T = sw1.T  # [DIM, 2048]
    sw3T = sw3.T
    sw2T = sw2.T  # [2048, DIM]
    for h in range(2):
        e = E + h
        sl = slice(h * INTER, (h + 1) * INTER)
        w13[e, :, :INTER] = sw1T[:, sl]
        w13[e, :, INTER:] = sw3T[:, sl]
        w2[e] = sw2T[sl, :]
        b1H[:, e * 8:(e + 1) * 8] = sb1[sl].reshape(8, P).T
        b3H[:, e * 8:(e + 1) * 8] = sb3[sl].reshape(8, P).T
    b2e = np.ascontiguousarray(eb2, dtype=f)
    sb2H = np.ascontiguousarray(sb2.reshape(KD, P).T, dtype=f)
    gwT = np.ascontiguousarray(gate_w.T)
    return dict(w13=w13, w2=w2, gwT=gwT, b1H=b1H, b3H=b3H, b2e=b2e, sb2H=sb2H)


def _make_in_maps(inputs):
    shared = _prep_shared(
        inputs["gate_w"], inputs["ew1"], inputs["eb1"], inputs["ew2"],
        inputs["eb2"], inputs["ew3"], inputs["eb3"], inputs["sw1"],
        inputs["sb1"], inputs["sw2"], inputs["sb2"], inputs["sw3"],
        inputs["sb3"])
    x = np.asarray(inputs["x"], np.float32)
    in_maps = []
    for c in range(NCORES):
        m = dict(shared)
        xs = x[c * TSH:(c + 1) * TSH, :]
        m["xT"] = np.ascontiguousarray(xs.T)
        if _CACHE.get("v2"):
            m["xrow"] = np.ascontiguousarray(xs)
        in_maps.append(m)
    return in_maps


def kernel(x, gate_w, ew1, eb1, ew2, eb2, ew3, eb3,
           sw1, sb1, sw2, sb2, sw3, sb3):
    from concourse import bass_utils

    nc = _get_nc()
    in_maps = _make_in_maps(dict(
        x=x, gate_w=gate_w, ew1=ew1, eb1=eb1, ew2=ew2, eb2=eb2, ew3=ew3,
        eb3=eb3, sw1=sw1, sb1=sb1, sw2=sw2, sb2=sb2, sw3=sw3, sb3=sb3))

    res = bass_utils.run_bass_kernel_spmd(
        nc, in_maps, core_ids=list(range(NCORES)), trace=False)

    out = np.empty((T, DIM), np.float32)
    for c in range(NCORES):
        if _CACHE.get("v2"):
            out[c * TSH:(c + 1) * TSH, :] = res.results[c]["outTok"]
        else:
            out[c * TSH:(c + 1) * TSH, :] = res.results[c]["out"].T
    return out


def time_kernel(inputs, iters=5):
    """Dev-only steady-state timing: build the sharded jitted executable once,
    keep inputs device-resident, time repeated executions."""
    import time

    import jax
    import jax.numpy as jnp
    from jax.sharding import Mesh, PartitionSpec
    from jax.experimental.shard_map import shard_map

    import concourse.mybir as mybir
    from concourse import bass2jax

    nc = _get_nc()
    in_maps = _make_in_maps(inputs)

    bass2jax.install_neuronx_cc_hook()

    part_name = nc.partition_id_tensor.name if nc.partition_id_tensor else None
    in_names, out_names, out_avals, zero_outs = [], [], [], []
    for alloc in nc.m.functions[0].allocations:
        if not isinstance(alloc, mybir.MemoryLocationSet):
            continue
        name = alloc.memorylocations[0].name
        if alloc.kind == "ExternalInput":
            if name != part_name:
                in_names.append(name)
        elif alloc.kind == "ExternalOutput":
            out_names.append(name)
            shape = tuple(alloc.tensor_shape)
            dtype = mybir.dt.np(alloc.dtype)
            out_avals.append(jax.core.ShapedArray(shape, dtype))
            zero_outs.append(np.zeros(shape, dtype))
    n_params = len(in_names)
    all_names = in_names + out_names
    if part_name is not None:
        all_names = all_names + [part_name]

    def _body(*args):
        operands = list(args)
        if part_name is not None:
            operands.append(bass2jax.partition_id_tensor())
        outs = bass2jax._bass_exec_p.bind(
            *operands,
            out_avals=tuple(out_avals),
            in_names=tuple(all_names),
            out_names=tuple(out_names),
            lowering_input_output_aliases=(),
            sim_require_finite=True,
            sim_require_nnan=True,
            nc=nc,
        )
        return tuple(outs)

    devices = jax.devices()[:NCORES]
    mesh = Mesh(np.asarray(devices), ("core",))
    in_specs = (PartitionSpec("core"),) * (n_params + len(out_names))
    out_specs = (PartitionSpec("core"),) * len(out_names)
    sharded = jax.jit(
        shard_map(_body, mesh=mesh, in_specs=in_specs, out_specs=out_specs,
                  check_rep=False),
        keep_unused=True,
    )
    concat_in = [
        np.concatenate([np.asarray(in_maps[c][n]) for c in range(NCORES)], axis=0)
        for n in in_names
    ]
    concat_zeros = [
        np.zeros((NCORES * z.shape[0], *z.shape[1:]), z.dtype) for z in zero_outs
    ]
    sharding = jax.sharding.NamedSharding(mesh, PartitionSpec("core"))
    dev_in = [jax.device_put(a, sharding) for a in concat_in]
    dev_zero = [jax.device_put(a, sharding) for a in concat_zeros]

    times = []
    out = sharded(*dev_in, *dev_zero)   # warmup/compile
    jax.block_until_ready(out)
    for _ in range(iters):
        t0 = time.perf_counter()
        out = sharded(*dev_in, *dev_zero)
        jax.block_until_ready(out)
        times.append(time.perf_counter() - t0)
    return times


def time_kernel_chained(inputs, chain=8, iters=3):
    """Chain `chain` kernel executions inside one jitted call, feeding the
    output back as xT.  Per-kernel time = slope between chain lengths."""
    import time

    import jax
    import jax.numpy as jnp
    from jax.sharding import Mesh, PartitionSpec
    from jax.experimental.shard_map import shard_map

    import concourse.mybir as mybir
    from concourse import bass2jax

    nc = _get_nc()
    in_maps = _make_in_maps(inputs)
    bass2jax.install_neuronx_cc_hook()

    part_name = nc.partition_id_tensor.name if nc.partition_id_tensor else None
    in_names, out_names, out_avals = [], [], []
    for alloc in nc.m.functions[0].allocations:
        if not isinstance(alloc, mybir.MemoryLocationSet):
            continue
        name = alloc.memorylocations[0].name
        if alloc.kind == "ExternalInput":
            if name != part_name:
                in_names.append(name)
        elif alloc.kind == "ExternalOutput":
            out_names.append(name)
            out_avals.append(jax.core.ShapedArray(
                tuple(alloc.tensor_shape), mybir.dt.np(alloc.dtype)))
    all_names = in_names + out_names
    if part_name is not None:
        all_names = all_names + [part_name]
    xt_pos = in_names.index("xT")

    def _one(args_by_name, zero_buf):
        operands = [args_by_name[n] for n in in_names]
        operands.append(zero_buf)
        if part_name is not None:
            operands.append(bass2jax.partition_id_tensor())
        outs = bass2jax._bass_exec_p.bind(
            *operands,
            out_avals=tuple(out_avals),
            in_names=tuple(all_names),
            out_names=tuple(out_names),
            lowering_input_output_aliases=(),
            sim_require_finite=True,
            sim_require_nnan=True,
            nc=nc,
        )
        return outs[0]

    def _chain_body(*args):
        d = dict(zip(in_names, args[:len(in_names)]))
        zeros = args[len(in_names):]
        out = _one(d, zeros[0])
        for j in range(chain - 1):
            d["xT"] = out
            out = _one(d, zeros[j + 1])
        return (out,)

    devices = jax.devices()[:NCORES]
    mesh = Mesh(np.asarray(devices), ("core",))
    in_specs = (PartitionSpec("core"),) * (len(in_names) + chain)
    out_specs = (PartitionSpec("core"),)
    sharded = jax.jit(
        shard_map(_chain_body, mesh=mesh, in_specs=in_specs,
                  out_specs=out_specs, check_rep=False),
        keep_unused=True,
    )
    concat_in = [
        np.concatenate([np.asarray(in_maps[c][n]) for c in range(NCORES)], axis=0)
        for n in in_names
    ]
    zshape = (NCORES * out_avals[0].shape[0], *out_avals[0].shape[1:])
    concat_in += [np.zeros(zshape, np.float32) for _ in range(chain)]
    sharding = jax.sharding.NamedSharding(mesh, PartitionSpec("core"))
    dev_in = [jax.device_put(a, sharding) for a in concat_in]

    out = sharded(*dev_in)
    jax.block_until_ready(out)
    times = []
    for _ in range(iters):
        t0 = time.perf_counter()
        out = sharded(*dev_in)
        jax.block_until_ready(out)
        times.append(time.perf_counter() - t0)
    return times
